# revision 21
# baseline (speedup 1.0000x reference)
"""DiffLogicLayer Trainium2 kernel — host-gather + u8-quantized 'a' operand with
weight-sensitivity routing, fp16 streaming, pair-fused TT ops, single sync ring.

Math: out = W0 + W1*a + W2*b + W3*ab per neuron, W = softmax(weights) @ C
(the 16 difflogic gates are linear in {1, a, b, ab}). softmax+C-fold is done
host-side in fp64 (weight preprocessing; 16KB/core of constants shipped).

Per block (128 neurons x 2048 batch): u = W3*a + W2, v = W1*a + W0,
t = u*b, o = t + v. Block flavors:

- F (fp16 'a'): u on ACT, v on DVE tensor_scalar (4x).
- A (u8 'a'): ScalarE reads u8 directly (ACTIVATE is 1x, dtype-independent)
  for BOTH u and v, /256 folded into scale; DVE does only t, o.
- D (u8 'a'): DVE unpacks byte pairs via uint16 bitwise ops into fp16
  y = 256 + q/4 (OR the byte into the mantissa of 0x5C00; and+or / shr+or,
  one tensor_scalar per half, ~418ns each); v = TS(y) 4x, u = ACT(y)
  contiguous; constants absorb the affine remap. Host packs D-block bytes
  interleaved (byte 2j = elem j, 2j+1 = elem H+j) so y lands in natural
  batch order — no strided reads anywhere.

t and o run as PAIR-FUSED TTs over adjacent blocks for pairs (2,3) and
(4,5) (u/v/b/t/o live in (128, 4096) pair tiles): TT carries no
per-partition scalars so fusing is legal, saving the 58-cycle + DRAIN
overhead per op. Pair (0,1) is UNFUSED (t0 needs only u0, ~2us before a
fused t01 could start) and emission order is hand-tuned from the trace:
blocks 0-3 prep first, block 6 prep before t45 (fills DVE's wait on ACT's
v4). The last pair (6,7) is UNFUSED and block 7 is computed in
batch-halves with two 256KB stores, so the final store drain overlaps the
last compute (o7 is the final DVE op; shrinking the post-o7 drain shortens
the kernel tail). Block 0/1 bytes + all 32 per-block fp32 constants ride in
ONE first DMA (a01c) — one completion-receipt latency (~2us) instead of two
gates the first compute; a23 loads before b01 to give DVE unpack work
during ScalarE's u0/v0/u1.

u8 'a' everywhere costs max rel err 1.9e-2 (gate 2e-2). Fix: neurons are
globally sorted by the weights-only sensitivity S = max(|W1|, |W1+W3|); the
top 1024 (12.5%) go to each core's single F block. Simulated: 5.8e-3.

Sharding: tensor-parallel over out_dim, host-chosen neuron permutation
(undone on host). ALL loads+stores on the sync HWDGE ring (FIFO; ~400GB/s
in-queue; split rings measured slower — v3/v14). GPSIMD unused (v5).
Contiguous full-tile DVE APs keep 2x/4x modes (v7). Per-core DMA: 6.3MB
loads + 4.2MB stores (vs 12.7MB all-fp16).
"""

import os
import sys

import numpy as np

sys.path.insert(0, "/opt/trn_rl_repo")

import concourse.bacc as bacc
import concourse.mybir as mybir
from concourse import tile
from concourse.bass_utils import run_bass_kernel_spmd

AF = mybir.ActivationFunctionType
ALU = mybir.AluOpType
F32 = mybir.dt.float32
F16 = mybir.dt.float16
U8 = mybir.dt.uint8
U16 = mybir.dt.uint16

IN_DIM = 8192
OUT_DIM = 8192
BATCH = 2048
N_CORES = 8
OPC = OUT_DIM // N_CORES
NBLK = OPC // 128
H = BATCH // 2

# Block flavors. Block 0 is A: ScalarE's first ops (u0, v0) read u8 directly
# and need no DVE unpack, so ACT starts the moment a01c's sem fires instead
# of idling behind the unpack. Blocks 1,2 are D so DVE has unpack/v work in
# the same window. The last pair (6,7) is (D,A): after b67 lands only
# t6/o6/t7/o7 remain on DVE and u7/v7 are ACT ops emitted last.
BLOCK_KIND = ["A", "D", "D", "A", "A", "F", "D", "A"]
FBLK = BLOCK_KIND.index("F")

_C = np.array(
    [
        [0, 0, 0, 0], [0, 0, 0, 1], [0, 1, 0, -1], [0, 1, 0, 0],
        [0, 0, 1, -1], [0, 0, 1, 0], [0, 1, 1, -2], [0, 1, 1, -1],
        [1, -1, -1, 1], [1, -1, -1, 2], [1, 0, -1, 0], [1, 0, -1, 1],
        [1, -1, 0, 0], [1, -1, 0, 1], [1, 0, 0, -1], [1, 0, 0, 0],
    ],
    dtype=np.float64,
)

_PROGRAM = None


def _build_program():
    nc = bacc.Bacc("TRN2", target_bir_lowering=False, debug=False)

    # a01c = blocks 0,1 u8 bytes + the 32 fp32 per-block constants, one DMA
    a01c = nc.dram_tensor("a01c", (128, 2 * BATCH + 16 * NBLK), U8, kind="ExternalInput")
    a23 = nc.dram_tensor("a23", (128, 2 * BATCH), U8, kind="ExternalInput")
    a4 = nc.dram_tensor("a4", (128, BATCH), U8, kind="ExternalInput")
    a5f = nc.dram_tensor("a5f", (128, BATCH), F16, kind="ExternalInput")
    a67 = nc.dram_tensor("a67", (128, 2 * BATCH), U8, kind="ExternalInput")
    bts = [
        nc.dram_tensor(f"b{p}{p + 1}", (128, 2 * BATCH), F16, kind="ExternalInput")
        for p in (0, 2, 4, 6)
    ]
    yts = [
        nc.dram_tensor(f"y{p}{p + 1}", (128, 2 * BATCH), F16, kind="ExternalOutput")
        for p in (0, 2, 4)
    ]
    # last pair stored small-to-smaller: y6, then two y7 quarter stores so
    # the final drain overlaps block 7's second half compute
    y6t = nc.dram_tensor("y6", (128, BATCH), F16, kind="ExternalOutput")
    y7at = nc.dram_tensor("y7a", (128, H), F16, kind="ExternalOutput")
    y7bt = nc.dram_tensor("y7b", (128, H), F16, kind="ExternalOutput")

    with tile.TileContext(nc) as tc:
        with (
            tc.tile_pool(name="const", bufs=1) as cpool,
            tc.tile_pool(name="gath", bufs=1) as gpool,
            tc.tile_pool(name="work", bufs=2) as wpool,
            tc.tile_pool(name="outp", bufs=1) as opool,
        ):
            # ---- loads: all on the sync HWDGE ring, in stream order ----
            a0c_t = gpool.tile([128, 2 * BATCH + 16 * NBLK], U8, tag="a01c")
            nc.sync.dma_start(a0c_t[:, :], a01c[:, :])
            b_t = [None] * 4
            # a23 before b01: fills DVE's early gap (unp2/v2) while t01 is
            # anyway gated on ScalarE's u1
            a23_t = gpool.tile([128, 2 * BATCH], U8, tag="a23")
            nc.sync.dma_start(a23_t[:, :], a23[:, :])
            b_t[0] = gpool.tile([128, 2 * BATCH], F16, name="b01t", tag="b01")
            nc.sync.dma_start(b_t[0][:, :], bts[0][:, :])
            b_t[1] = gpool.tile([128, 2 * BATCH], F16, name="b23t", tag="b23")
            nc.sync.dma_start(b_t[1][:, :], bts[1][:, :])
            a4_t = gpool.tile([128, BATCH], U8, tag="a4")
            nc.sync.dma_start(a4_t[:, :], a4[:, :])
            a5f_t = gpool.tile([128, BATCH], F16, tag="a5f")
            nc.sync.dma_start(a5f_t[:, :], a5f[:, :])
            b_t[2] = gpool.tile([128, 2 * BATCH], F16, name="b45t", tag="b45")
            nc.sync.dma_start(b_t[2][:, :], bts[2][:, :])
            a67_t = gpool.tile([128, 2 * BATCH], U8, tag="a67")
            nc.sync.dma_start(a67_t[:, :], a67[:, :])
            b_t[3] = gpool.tile([128, 2 * BATCH], F16, name="b67t", tag="b67")
            nc.sync.dma_start(b_t[3][:, :], bts[3][:, :])

            def a8_ap(j):
                if j in (0, 1):
                    off = j * BATCH
                    return a0c_t[:, off : off + BATCH]
                src = {2: a23_t, 3: a23_t, 4: a4_t, 6: a67_t, 7: a67_t}[j]
                if j == 4:
                    return src[:, :]
                off = (j % 2) * BATCH
                return src[:, off : off + BATCH]

            # per-block constants live as 16 bytes each at the tail of a01c
            def _csf(j, k):
                off = 2 * BATCH + 16 * j + 4 * k
                return a0c_t[:, off : off + 4].bitcast(F32)

            def su(j):
                return _csf(j, 0)

            def bu(j):
                return _csf(j, 1)

            def sv(j):
                return _csf(j, 2)

            def bv(j):
                return _csf(j, 3)

            o_t = [
                opool.tile([128, 2 * BATCH], F16, name=f"o{p}{p + 1}", tag=f"o{p}")
                for p in (0, 2, 4)
            ]
            o6_t = opool.tile([128, BATCH], F16, tag="o6")
            o7_t = opool.tile([128, BATCH], F16, tag="o7")

            pair_uvt = {}
            for pi in range(3):
                pair_uvt[pi] = (
                    wpool.tile([128, 2 * BATCH], F16, name=f"u{pi}", tag="u"),
                    wpool.tile([128, 2 * BATCH], F16, name=f"v{pi}", tag="v"),
                    wpool.tile([128, 2 * BATCH], F16, name=f"t{pi}", tag="t"),
                )

            def prep(j):
                # emit u/v (+unpack) for block j into its pair tiles
                pi, half = j // 2, j % 2
                u_t, v_t, _ = pair_uvt[pi]
                lo, hi = half * BATCH, (half + 1) * BATCH
                u_ap = u_t[:, lo:hi]
                v_ap = v_t[:, lo:hi]
                kind = BLOCK_KIND[j]
                if kind == "F":
                    a_ap = a5f_t[:, :]
                    nc.scalar.activation(
                        u_ap, a_ap, AF.Identity, bias=bu(j), scale=su(j)
                    )
                    nc.vector.tensor_scalar(
                        v_ap, a_ap, sv(j), bv(j), op0=ALU.mult, op1=ALU.add
                    )
                elif kind == "A":
                    a_ap = a8_ap(j)
                    nc.scalar.activation(
                        u_ap, a_ap, AF.Identity, bias=bu(j), scale=su(j)
                    )
                    nc.scalar.activation(
                        v_ap, a_ap, AF.Identity, bias=bv(j), scale=sv(j)
                    )
                else:  # D
                    w16 = a8_ap(j).bitcast(U16)
                    y_t = wpool.tile([128, BATCH], F16, name=f"y{j}d", tag=f"y{j}d")
                    nc.vector.tensor_scalar(
                        y_t[:, 0:H].bitcast(U16), w16, 0x00FF, 0x5C00,
                        op0=ALU.bitwise_and, op1=ALU.bitwise_or,
                    )
                    nc.vector.tensor_scalar(
                        y_t[:, H:BATCH].bitcast(U16), w16, 8, 0x5C00,
                        op0=ALU.logical_shift_right, op1=ALU.bitwise_or,
                    )
                    nc.scalar.activation(
                        u_ap, y_t[:, :], AF.Identity, bias=bu(j), scale=su(j)
                    )
                    nc.vector.tensor_scalar(
                        v_ap, y_t[:, :], sv(j), bv(j), op0=ALU.mult, op1=ALU.add
                    )

            def to_pair(pi):
                u_t, v_t, t_t = pair_uvt[pi]
                nc.vector.tensor_tensor(
                    t_t[:, :], u_t[:, :], b_t[pi][:, :], op=ALU.mult
                )
                nc.vector.tensor_tensor(
                    o_t[pi][:, :], t_t[:, :], v_t[:, :], op=ALU.add
                )

            def to_half(j):
                pi, half = j // 2, j % 2
                u_t, v_t, t_t = pair_uvt[pi]
                lo, hi = half * BATCH, (half + 1) * BATCH
                nc.vector.tensor_tensor(
                    t_t[:, lo:hi], u_t[:, lo:hi], b_t[pi][:, lo:hi], op=ALU.mult
                )
                nc.vector.tensor_tensor(
                    o_t[pi][:, lo:hi], t_t[:, lo:hi], v_t[:, lo:hi], op=ALU.add
                )

            # Emission order tuned from trace: blocks 0-3 prep first so DVE has
            # unpack/v work while ScalarE produces u0/v0/u1; pair 0 t/o UNFUSED
            # (t0 needs only u0, ~2us before the fused t01 could start); block
            # 6 prep pulled before t45 to fill DVE's wait on ACT's v4.
            prep(0)
            prep(1)
            prep(2)
            prep(3)
            to_half(0)
            to_half(1)
            to_pair(1)
            prep(4)
            prep(5)

            # ---- last pair (6=D, 7=A), unfused so y6 stores while 7 finishes
            u6_t = wpool.tile([128, BATCH], F16, tag="u6")
            v6_t = wpool.tile([128, BATCH], F16, tag="v6")
            t6_t = wpool.tile([128, BATCH], F16, tag="t6")
            w16_6 = a8_ap(6).bitcast(U16)
            y6d_t = wpool.tile([128, BATCH], F16, tag="y6d")
            nc.vector.tensor_scalar(
                y6d_t[:, 0:H].bitcast(U16), w16_6, 0x00FF, 0x5C00,
                op0=ALU.bitwise_and, op1=ALU.bitwise_or,
            )
            nc.vector.tensor_scalar(
                y6d_t[:, H:BATCH].bitcast(U16), w16_6, 8, 0x5C00,
                op0=ALU.logical_shift_right, op1=ALU.bitwise_or,
            )
            nc.scalar.activation(
                u6_t[:, :], y6d_t[:, :], AF.Identity, bias=bu(6), scale=su(6)
            )
            nc.vector.tensor_scalar(
                v6_t[:, :], y6d_t[:, :], sv(6), bv(6), op0=ALU.mult, op1=ALU.add
            )
            u7_t = wpool.tile([128, BATCH], F16, tag="u7")
            v7_t = wpool.tile([128, BATCH], F16, tag="v7")
            t7_t = wpool.tile([128, BATCH], F16, tag="t7")
            a7_ap = a8_ap(7)
            nc.scalar.activation(
                u7_t[:, :], a7_ap, AF.Identity, bias=bu(7), scale=su(7)
            )
            nc.scalar.activation(
                v7_t[:, :], a7_ap, AF.Identity, bias=bv(7), scale=sv(7)
            )
            # pair (4,5) t/o emitted after block 6/7 prep: DVE fills its wait
            # on ACT's v4 with unpack6/v6 work
            to_pair(2)
            b67 = b_t[3]
            nc.vector.tensor_tensor(t6_t[:, :], u6_t[:, :], b67[:, 0:BATCH], op=ALU.mult)
            nc.vector.tensor_tensor(o6_t[:, :], t6_t[:, :], v6_t[:, :], op=ALU.add)
            for hh in range(2):
                lo, hi = BATCH + hh * H, BATCH + (hh + 1) * H
                nc.vector.tensor_tensor(
                    t7_t[:, hh * H : (hh + 1) * H], u7_t[:, hh * H : (hh + 1) * H],
                    b67[:, lo:hi], op=ALU.mult,
                )
                nc.vector.tensor_tensor(
                    o7_t[:, hh * H : (hh + 1) * H], t7_t[:, hh * H : (hh + 1) * H],
                    v7_t[:, hh * H : (hh + 1) * H], op=ALU.add,
                )

            # ---- stores: SAME sync ring, queued behind all loads (FIFO) ----
            for pi in range(3):
                nc.sync.dma_start(yts[pi][:, :], o_t[pi][:, :])
            nc.sync.dma_start(y6t[:, :], o6_t[:, :])
            nc.sync.dma_start(y7at[:, :], o7_t[:, 0:H])
            nc.sync.dma_start(y7bt[:, :], o7_t[:, H:BATCH])

    nc.compile()
    return nc


def _get_program():
    global _PROGRAM
    if _PROGRAM is None:
        _PROGRAM = _build_program()
    return _PROGRAM


def make_in_maps(x, weights, indices_a, indices_b):
    x = np.asarray(x, dtype=np.float32)
    w = np.asarray(weights, dtype=np.float64)
    ia = np.asarray(indices_a).astype(np.int64)
    ib = np.asarray(indices_b).astype(np.int64)

    e = np.exp(w - w.max(axis=1, keepdims=True))
    sm = e / e.sum(axis=1, keepdims=True)
    W = sm @ _C  # (OUT_DIM, 4): W0..W3

    S = np.maximum(np.abs(W[:, 1]), np.abs(W[:, 1] + W[:, 3]))
    order = np.argsort(-S, kind="stable")
    sens, rest = order[:1024], order[1024:]
    nperm = np.empty((N_CORES, NBLK, 128), dtype=np.int64)
    ri = 0
    for c in range(N_CORES):
        nperm[c, FBLK] = sens[c * 128 : (c + 1) * 128]
        for j in range(NBLK):
            if j == FBLK:
                continue
            nperm[c, j] = rest[ri : ri + 128]
            ri += 128

    xt16 = np.ascontiguousarray(x.T.astype(np.float16))  # (IN_DIM, BATCH)
    xt8 = np.clip(np.round(x.T * np.float32(256.0)), 0, 255).astype(np.uint8)

    in_maps = []
    for c in range(N_CORES):
        cs = np.empty((128, 4 * NBLK), dtype=np.float32)
        m = {}
        ga8 = {}
        for j in range(NBLK):
            nid = nperm[c, j]
            W0, W1, W2, W3 = (W[nid, k] for k in range(4))
            kind = BLOCK_KIND[j]
            if kind == "F":
                su, bu, sv, bv = W3, W2, W1, W0
                m["a5f"] = np.ascontiguousarray(xt16[ia[nid]])
            elif kind == "A":
                su, bu, sv, bv = W3 / 256.0, W2, W1 / 256.0, W0
                ga8[j] = xt8[ia[nid]]
            else:  # D: y = 256 + q/4 -> q = 4*(y-256)
                su, bu = W3 / 64.0, W2 - 4.0 * W3
                sv, bv = W1 / 64.0, W0 - 4.0 * W1
                q = xt8[ia[nid]]
                il = np.empty((128, BATCH), dtype=np.uint8)
                il[:, 0::2] = q[:, :H]
                il[:, 1::2] = q[:, H:]
                ga8[j] = il
            cs[:, 4 * j + 0] = su
            cs[:, 4 * j + 1] = bu
            cs[:, 4 * j + 2] = sv
            cs[:, 4 * j + 3] = bv
        m["a4"] = np.ascontiguousarray(ga8[4])
        m["a01c"] = np.ascontiguousarray(
            np.concatenate([ga8[0], ga8[1], cs.view(np.uint8)], axis=1)
        )
        for pair in ((2, 3), (6, 7)):
            m[f"a{pair[0]}{pair[1]}"] = np.ascontiguousarray(
                np.concatenate([ga8[pair[0]], ga8[pair[1]]], axis=1)
            )
        for p in (0, 2, 4, 6):
            blk = np.empty((128, 2, BATCH), dtype=np.float16)
            blk[:, 0, :] = xt16[ib[nperm[c, p]]]
            blk[:, 1, :] = xt16[ib[nperm[c, p + 1]]]
            m[f"b{p}{p + 1}"] = np.ascontiguousarray(blk.reshape(128, 2 * BATCH))
        in_maps.append(m)
    return in_maps, nperm


def run(inputs, trace=False):
    if trace:
        try:
            from antenv.axon_hooks import get_axon_ntff_profile_hook  # noqa: F401
        except ImportError:
            trace = False
    nc = _get_program()
    in_maps, nperm = make_in_maps(
        inputs["x"], inputs["weights"], inputs["indices_a"], inputs["indices_b"]
    )
    res = run_bass_kernel_spmd(nc, in_maps, core_ids=list(range(N_CORES)), trace=trace)
    outT = np.empty((OUT_DIM, BATCH), dtype=np.float32)
    for c in range(N_CORES):
        r = res.results[c]
        for p in (0, 2, 4):
            pair = r[f"y{p}{p + 1}"].astype(np.float32)
            outT[nperm[c, p]] = pair[:, :BATCH]
            outT[nperm[c, p + 1]] = pair[:, BATCH:]
        outT[nperm[c, 6]] = r["y6"].astype(np.float32)
        outT[nperm[c, 7]] = np.concatenate(
            [r["y7a"], r["y7b"]], axis=1
        ).astype(np.float32)
    return np.ascontiguousarray(outT.T), res


def kernel(**inputs):
    out, _ = run(inputs, trace=bool(os.environ.get("DL_TRACE")))
    return out


if __name__ == "__main__":
    rng = np.random.default_rng(0)
    inputs = {
        "x": rng.random((BATCH, IN_DIM), dtype=np.float32),
        "weights": rng.standard_normal((OUT_DIM, 16)).astype(np.float32),
        "indices_a": rng.integers(0, IN_DIM, size=OUT_DIM),
        "indices_b": rng.integers(0, IN_DIM, size=OUT_DIM),
    }
    out = kernel(**inputs)
    print(out.shape, out.dtype)


# revision 22
# speedup vs baseline: 1.1462x; 1.1462x over previous
"""DiffLogicLayer Trainium2 kernel — host-gather + u8-quantized 'a' operand with
weight-sensitivity routing, fp16 streaming, pair-fused TT ops, single sync ring.

Math: out = W0 + W1*a + W2*b + W3*ab per neuron, W = softmax(weights) @ C
(the 16 difflogic gates are linear in {1, a, b, ab}). softmax+C-fold is done
host-side in fp64 (weight preprocessing; 16KB/core of constants shipped).

Per block (128 neurons x 2048 batch): u = W3*a + W2, v = W1*a + W0,
t = u*b, o = t + v. Block flavors:

- F (fp16 'a'): u on ACT, v on DVE tensor_scalar (4x).
- A (u8 'a'): ScalarE reads u8 directly (ACTIVATE is 1x, dtype-independent)
  for BOTH u and v, /256 folded into scale; DVE does only t, o.
- D (u8 'a'): DVE unpacks byte pairs via uint16 bitwise ops into fp16
  y = 256 + q/4 (OR the byte into the mantissa of 0x5C00; and+or / shr+or,
  one tensor_scalar per half, ~418ns each); v = TS(y) 4x, u = ACT(y)
  contiguous; constants absorb the affine remap. Host packs D-block bytes
  interleaved (byte 2j = elem j, 2j+1 = elem H+j) so y lands in natural
  batch order — no strided reads anywhere.

t and o run as PAIR-FUSED TTs over adjacent blocks for pairs (2,3) and
(4,5) (u/v/b/t/o live in (128, 4096) pair tiles): TT carries no
per-partition scalars so fusing is legal, saving the 58-cycle + DRAIN
overhead per op. Pair (0,1) is UNFUSED (t0 needs only u0, ~2us before a
fused t01 could start) and emission order is hand-tuned from the trace:
blocks 0-3 prep first, block 6 prep before t45 (fills DVE's wait on ACT's
v4). The last pair (6,7) is UNFUSED and block 7 is computed in
batch-halves with two 256KB stores, so the final store drain overlaps the
last compute (o7 is the final DVE op; shrinking the post-o7 drain shortens
the kernel tail). Block 0/1 bytes + all 32 per-block fp32 constants ride in
ONE first DMA (a01c) — one completion-receipt latency (~2us) instead of two
gates the first compute; a23 loads before b01 to give DVE unpack work
during ScalarE's u0/v0/u1.

u8 'a' everywhere costs max rel err 1.9e-2 (gate 2e-2). Fix: neurons are
globally sorted by the weights-only sensitivity S = max(|W1|, |W1+W3|); the
top 1024 (12.5%) go to each core's single F block. Simulated: 5.8e-3.

Sharding: tensor-parallel over out_dim, host-chosen neuron permutation
(undone on host). ALL loads+stores on the sync HWDGE ring (FIFO; ~400GB/s
in-queue; split rings measured slower — v3/v14). GPSIMD unused (v5).
Contiguous full-tile DVE APs keep 2x/4x modes (v7). Per-core DMA: 6.3MB
loads + 4.2MB stores (vs 12.7MB all-fp16).
"""

import os
import sys

import numpy as np

sys.path.insert(0, "/opt/trn_rl_repo")

import concourse.bacc as bacc
import concourse.mybir as mybir
from concourse import tile
from concourse.bass_utils import run_bass_kernel_spmd

AF = mybir.ActivationFunctionType
ALU = mybir.AluOpType
F32 = mybir.dt.float32
F16 = mybir.dt.float16
U8 = mybir.dt.uint8
U16 = mybir.dt.uint16

IN_DIM = 8192
OUT_DIM = 8192
BATCH = 2048
N_CORES = 8
OPC = OUT_DIM // N_CORES
NBLK = OPC // 128
H = BATCH // 2

# Block flavors. Block 0 is A: ScalarE's first ops (u0, v0) read u8 directly
# and need no DVE unpack, so ACT starts the moment a01c's sem fires instead
# of idling behind the unpack. Blocks 1,2 are D so DVE has unpack/v work in
# the same window. The last pair (6,7) is (D,A): after b67 lands only
# t6/o6/t7/o7 remain on DVE and u7/v7 are ACT ops emitted last.
BLOCK_KIND = ["A", "D", "D", "A", "A", "F", "D", "A"]
FBLK = BLOCK_KIND.index("F")

_C = np.array(
    [
        [0, 0, 0, 0], [0, 0, 0, 1], [0, 1, 0, -1], [0, 1, 0, 0],
        [0, 0, 1, -1], [0, 0, 1, 0], [0, 1, 1, -2], [0, 1, 1, -1],
        [1, -1, -1, 1], [1, -1, -1, 2], [1, 0, -1, 0], [1, 0, -1, 1],
        [1, -1, 0, 0], [1, -1, 0, 1], [1, 0, 0, -1], [1, 0, 0, 0],
    ],
    dtype=np.float64,
)

_PROGRAM = None


def _build_program():
    nc = bacc.Bacc("TRN2", target_bir_lowering=False, debug=False)

    # a01c = blocks 0,1 u8 bytes + the 32 fp32 per-block constants, one DMA
    a01c = nc.dram_tensor("a01c", (128, 2 * BATCH + 16 * NBLK), U8, kind="ExternalInput")
    a23 = nc.dram_tensor("a23", (128, 2 * BATCH), U8, kind="ExternalInput")
    a4 = nc.dram_tensor("a4", (128, BATCH), U8, kind="ExternalInput")
    a5f = nc.dram_tensor("a5f", (128, BATCH), F16, kind="ExternalInput")
    a67 = nc.dram_tensor("a67", (128, 2 * BATCH), U8, kind="ExternalInput")
    bts = [
        nc.dram_tensor(f"b{p}{p + 1}", (128, 2 * BATCH), F16, kind="ExternalInput")
        for p in (0, 2, 4, 6)
    ]
    yts = [
        nc.dram_tensor(f"y{p}{p + 1}", (128, 2 * BATCH), F16, kind="ExternalOutput")
        for p in (0, 2, 4)
    ]
    # last pair stored small-to-smaller: y6, then two y7 quarter stores so
    # the final drain overlaps block 7's second half compute
    y6t = nc.dram_tensor("y6", (128, BATCH), F16, kind="ExternalOutput")
    y7at = nc.dram_tensor("y7a", (128, H), F16, kind="ExternalOutput")
    y7bt = nc.dram_tensor("y7b", (128, H), F16, kind="ExternalOutput")

    with tile.TileContext(nc) as tc:
        with (
            tc.tile_pool(name="const", bufs=1) as cpool,
            tc.tile_pool(name="gath", bufs=1) as gpool,
            tc.tile_pool(name="work", bufs=2) as wpool,
            tc.tile_pool(name="outp", bufs=1) as opool,
        ):
            # ---- loads: all on the sync HWDGE ring, in stream order ----
            a0c_t = gpool.tile([128, 2 * BATCH + 16 * NBLK], U8, tag="a01c")
            nc.sync.dma_start(a0c_t[:, :], a01c[:, :])
            b_t = [None] * 4
            # a23 before b01: fills DVE's early gap (unp2/v2) while t01 is
            # anyway gated on ScalarE's u1
            a23_t = gpool.tile([128, 2 * BATCH], U8, tag="a23")
            nc.sync.dma_start(a23_t[:, :], a23[:, :])
            b_t[0] = gpool.tile([128, 2 * BATCH], F16, name="b01t", tag="b01")
            nc.sync.dma_start(b_t[0][:, :], bts[0][:, :])
            b_t[1] = gpool.tile([128, 2 * BATCH], F16, name="b23t", tag="b23")
            nc.sync.dma_start(b_t[1][:, :], bts[1][:, :])
            a4_t = gpool.tile([128, BATCH], U8, tag="a4")
            nc.sync.dma_start(a4_t[:, :], a4[:, :])
            a5f_t = gpool.tile([128, BATCH], F16, tag="a5f")
            nc.sync.dma_start(a5f_t[:, :], a5f[:, :])
            b_t[2] = gpool.tile([128, 2 * BATCH], F16, name="b45t", tag="b45")
            nc.sync.dma_start(b_t[2][:, :], bts[2][:, :])
            a67_t = gpool.tile([128, 2 * BATCH], U8, tag="a67")
            nc.sync.dma_start(a67_t[:, :], a67[:, :])
            b_t[3] = gpool.tile([128, 2 * BATCH], F16, name="b67t", tag="b67")
            nc.sync.dma_start(b_t[3][:, :], bts[3][:, :])

            def a8_ap(j):
                if j in (0, 1):
                    off = j * BATCH
                    return a0c_t[:, off : off + BATCH]
                src = {2: a23_t, 3: a23_t, 4: a4_t, 6: a67_t, 7: a67_t}[j]
                if j == 4:
                    return src[:, :]
                off = (j % 2) * BATCH
                return src[:, off : off + BATCH]

            # per-block constants live as 16 bytes each at the tail of a01c
            def _csf(j, k):
                off = 2 * BATCH + 16 * j + 4 * k
                return a0c_t[:, off : off + 4].bitcast(F32)

            def su(j):
                return _csf(j, 0)

            def bu(j):
                return _csf(j, 1)

            def sv(j):
                return _csf(j, 2)

            def bv(j):
                return _csf(j, 3)

            o_t = [
                opool.tile([128, 2 * BATCH], F16, name=f"o{p}{p + 1}", tag=f"o{p}")
                for p in (0, 2, 4)
            ]
            o6_t = opool.tile([128, BATCH], F16, tag="o6")
            o7_t = opool.tile([128, BATCH], F16, tag="o7")

            pair_uvt = {}
            for pi in range(3):
                pair_uvt[pi] = (
                    wpool.tile([128, 2 * BATCH], F16, name=f"u{pi}", tag="u"),
                    wpool.tile([128, 2 * BATCH], F16, name=f"v{pi}", tag="v"),
                    wpool.tile([128, 2 * BATCH], F16, name=f"t{pi}", tag="t"),
                )

            def prep(j):
                # emit u/v (+unpack) for block j into its pair tiles
                pi, half = j // 2, j % 2
                u_t, v_t, _ = pair_uvt[pi]
                lo, hi = half * BATCH, (half + 1) * BATCH
                u_ap = u_t[:, lo:hi]
                v_ap = v_t[:, lo:hi]
                kind = BLOCK_KIND[j]
                if kind == "F":
                    a_ap = a5f_t[:, :]
                    nc.scalar.activation(
                        u_ap, a_ap, AF.Identity, bias=bu(j), scale=su(j)
                    )
                    nc.vector.tensor_scalar(
                        v_ap, a_ap, sv(j), bv(j), op0=ALU.mult, op1=ALU.add
                    )
                elif kind == "A":
                    a_ap = a8_ap(j)
                    nc.scalar.activation(
                        u_ap, a_ap, AF.Identity, bias=bu(j), scale=su(j)
                    )
                    nc.scalar.activation(
                        v_ap, a_ap, AF.Identity, bias=bv(j), scale=sv(j)
                    )
                else:  # D
                    w16 = a8_ap(j).bitcast(U16)
                    y_t = wpool.tile([128, BATCH], F16, name=f"y{j}d", tag=f"y{j}d")
                    nc.vector.tensor_scalar(
                        y_t[:, 0:H].bitcast(U16), w16, 0x00FF, 0x5C00,
                        op0=ALU.bitwise_and, op1=ALU.bitwise_or,
                    )
                    nc.vector.tensor_scalar(
                        y_t[:, H:BATCH].bitcast(U16), w16, 8, 0x5C00,
                        op0=ALU.logical_shift_right, op1=ALU.bitwise_or,
                    )
                    nc.scalar.activation(
                        u_ap, y_t[:, :], AF.Identity, bias=bu(j), scale=su(j)
                    )
                    nc.vector.tensor_scalar(
                        v_ap, y_t[:, :], sv(j), bv(j), op0=ALU.mult, op1=ALU.add
                    )

            def to_pair(pi):
                u_t, v_t, t_t = pair_uvt[pi]
                nc.vector.tensor_tensor(
                    t_t[:, :], u_t[:, :], b_t[pi][:, :], op=ALU.mult
                )
                nc.vector.tensor_tensor(
                    o_t[pi][:, :], t_t[:, :], v_t[:, :], op=ALU.add
                )

            def to_half(j):
                pi, half = j // 2, j % 2
                u_t, v_t, t_t = pair_uvt[pi]
                lo, hi = half * BATCH, (half + 1) * BATCH
                nc.vector.tensor_tensor(
                    t_t[:, lo:hi], u_t[:, lo:hi], b_t[pi][:, lo:hi], op=ALU.mult
                )
                nc.vector.tensor_tensor(
                    o_t[pi][:, lo:hi], t_t[:, lo:hi], v_t[:, lo:hi], op=ALU.add
                )

            # Emission order tuned from trace: blocks 0-3 prep first so DVE has
            # unpack/v work while ScalarE produces u0/v0/u1; pair 0 t/o UNFUSED
            # (t0 needs only u0, ~2us before the fused t01 could start); block
            # 6 prep pulled before t45 to fill DVE's wait on ACT's v4.
            prep(0)
            prep(1)
            prep(2)
            prep(3)
            to_half(0)
            to_half(1)
            to_pair(1)
            prep(4)
            prep(5)

            # ---- last pair (6=D, 7=A), unfused so y6 stores while 7 finishes
            u6_t = wpool.tile([128, BATCH], F16, tag="u6")
            v6_t = wpool.tile([128, BATCH], F16, tag="v6")
            t6_t = wpool.tile([128, BATCH], F16, tag="t6")
            w16_6 = a8_ap(6).bitcast(U16)
            y6d_t = wpool.tile([128, BATCH], F16, tag="y6d")
            nc.vector.tensor_scalar(
                y6d_t[:, 0:H].bitcast(U16), w16_6, 0x00FF, 0x5C00,
                op0=ALU.bitwise_and, op1=ALU.bitwise_or,
            )
            nc.vector.tensor_scalar(
                y6d_t[:, H:BATCH].bitcast(U16), w16_6, 8, 0x5C00,
                op0=ALU.logical_shift_right, op1=ALU.bitwise_or,
            )
            nc.scalar.activation(
                u6_t[:, :], y6d_t[:, :], AF.Identity, bias=bu(6), scale=su(6)
            )
            nc.vector.tensor_scalar(
                v6_t[:, :], y6d_t[:, :], sv(6), bv(6), op0=ALU.mult, op1=ALU.add
            )
            u7_t = wpool.tile([128, BATCH], F16, tag="u7")
            v7_t = wpool.tile([128, BATCH], F16, tag="v7")
            t7_t = wpool.tile([128, BATCH], F16, tag="t7")
            a7_ap = a8_ap(7)
            nc.scalar.activation(
                u7_t[:, :], a7_ap, AF.Identity, bias=bu(7), scale=su(7)
            )
            nc.scalar.activation(
                v7_t[:, :], a7_ap, AF.Identity, bias=bv(7), scale=sv(7)
            )
            # pair (4,5) t/o emitted after block 6/7 prep: DVE fills its wait
            # on ACT's v4 with unpack6/v6 work
            to_pair(2)
            b67 = b_t[3]
            nc.vector.tensor_tensor(t6_t[:, :], u6_t[:, :], b67[:, 0:BATCH], op=ALU.mult)
            nc.vector.tensor_tensor(o6_t[:, :], t6_t[:, :], v6_t[:, :], op=ALU.add)
            for hh in range(2):
                lo, hi = BATCH + hh * H, BATCH + (hh + 1) * H
                nc.vector.tensor_tensor(
                    t7_t[:, hh * H : (hh + 1) * H], u7_t[:, hh * H : (hh + 1) * H],
                    b67[:, lo:hi], op=ALU.mult,
                )
                nc.vector.tensor_tensor(
                    o7_t[:, hh * H : (hh + 1) * H], t7_t[:, hh * H : (hh + 1) * H],
                    v7_t[:, hh * H : (hh + 1) * H], op=ALU.add,
                )

            # ---- stores: scalar HWDGE ring, overlapping the sync ring's
            # loads (early o's are ready mid-load-stream; cold-start only
            # penalizes the first store, which is not on the critical tail)
            for pi in range(3):
                nc.scalar.dma_start(yts[pi][:, :], o_t[pi][:, :])
            nc.scalar.dma_start(y6t[:, :], o6_t[:, :])
            nc.scalar.dma_start(y7at[:, :], o7_t[:, 0:H])
            nc.scalar.dma_start(y7bt[:, :], o7_t[:, H:BATCH])

    nc.compile()
    return nc


def _get_program():
    global _PROGRAM
    if _PROGRAM is None:
        _PROGRAM = _build_program()
    return _PROGRAM


def make_in_maps(x, weights, indices_a, indices_b):
    x = np.asarray(x, dtype=np.float32)
    w = np.asarray(weights, dtype=np.float64)
    ia = np.asarray(indices_a).astype(np.int64)
    ib = np.asarray(indices_b).astype(np.int64)

    e = np.exp(w - w.max(axis=1, keepdims=True))
    sm = e / e.sum(axis=1, keepdims=True)
    W = sm @ _C  # (OUT_DIM, 4): W0..W3

    S = np.maximum(np.abs(W[:, 1]), np.abs(W[:, 1] + W[:, 3]))
    order = np.argsort(-S, kind="stable")
    sens, rest = order[:1024], order[1024:]
    nperm = np.empty((N_CORES, NBLK, 128), dtype=np.int64)
    ri = 0
    for c in range(N_CORES):
        nperm[c, FBLK] = sens[c * 128 : (c + 1) * 128]
        for j in range(NBLK):
            if j == FBLK:
                continue
            nperm[c, j] = rest[ri : ri + 128]
            ri += 128

    xt16 = np.ascontiguousarray(x.T.astype(np.float16))  # (IN_DIM, BATCH)
    xt8 = np.clip(np.round(x.T * np.float32(256.0)), 0, 255).astype(np.uint8)

    in_maps = []
    for c in range(N_CORES):
        cs = np.empty((128, 4 * NBLK), dtype=np.float32)
        m = {}
        ga8 = {}
        for j in range(NBLK):
            nid = nperm[c, j]
            W0, W1, W2, W3 = (W[nid, k] for k in range(4))
            kind = BLOCK_KIND[j]
            if kind == "F":
                su, bu, sv, bv = W3, W2, W1, W0
                m["a5f"] = np.ascontiguousarray(xt16[ia[nid]])
            elif kind == "A":
                su, bu, sv, bv = W3 / 256.0, W2, W1 / 256.0, W0
                ga8[j] = xt8[ia[nid]]
            else:  # D: y = 256 + q/4 -> q = 4*(y-256)
                su, bu = W3 / 64.0, W2 - 4.0 * W3
                sv, bv = W1 / 64.0, W0 - 4.0 * W1
                q = xt8[ia[nid]]
                il = np.empty((128, BATCH), dtype=np.uint8)
                il[:, 0::2] = q[:, :H]
                il[:, 1::2] = q[:, H:]
                ga8[j] = il
            cs[:, 4 * j + 0] = su
            cs[:, 4 * j + 1] = bu
            cs[:, 4 * j + 2] = sv
            cs[:, 4 * j + 3] = bv
        m["a4"] = np.ascontiguousarray(ga8[4])
        m["a01c"] = np.ascontiguousarray(
            np.concatenate([ga8[0], ga8[1], cs.view(np.uint8)], axis=1)
        )
        for pair in ((2, 3), (6, 7)):
            m[f"a{pair[0]}{pair[1]}"] = np.ascontiguousarray(
                np.concatenate([ga8[pair[0]], ga8[pair[1]]], axis=1)
            )
        for p in (0, 2, 4, 6):
            blk = np.empty((128, 2, BATCH), dtype=np.float16)
            blk[:, 0, :] = xt16[ib[nperm[c, p]]]
            blk[:, 1, :] = xt16[ib[nperm[c, p + 1]]]
            m[f"b{p}{p + 1}"] = np.ascontiguousarray(blk.reshape(128, 2 * BATCH))
        in_maps.append(m)
    return in_maps, nperm


def run(inputs, trace=False):
    if trace:
        try:
            from antenv.axon_hooks import get_axon_ntff_profile_hook  # noqa: F401
        except ImportError:
            trace = False
    nc = _get_program()
    in_maps, nperm = make_in_maps(
        inputs["x"], inputs["weights"], inputs["indices_a"], inputs["indices_b"]
    )
    res = run_bass_kernel_spmd(nc, in_maps, core_ids=list(range(N_CORES)), trace=trace)
    outT = np.empty((OUT_DIM, BATCH), dtype=np.float32)
    for c in range(N_CORES):
        r = res.results[c]
        for p in (0, 2, 4):
            pair = r[f"y{p}{p + 1}"].astype(np.float32)
            outT[nperm[c, p]] = pair[:, :BATCH]
            outT[nperm[c, p + 1]] = pair[:, BATCH:]
        outT[nperm[c, 6]] = r["y6"].astype(np.float32)
        outT[nperm[c, 7]] = np.concatenate(
            [r["y7a"], r["y7b"]], axis=1
        ).astype(np.float32)
    return np.ascontiguousarray(outT.T), res


def kernel(**inputs):
    out, _ = run(inputs, trace=bool(os.environ.get("DL_TRACE")))
    return out


if __name__ == "__main__":
    rng = np.random.default_rng(0)
    inputs = {
        "x": rng.random((BATCH, IN_DIM), dtype=np.float32),
        "weights": rng.standard_normal((OUT_DIM, 16)).astype(np.float32),
        "indices_a": rng.integers(0, IN_DIM, size=OUT_DIM),
        "indices_b": rng.integers(0, IN_DIM, size=OUT_DIM),
    }
    out = kernel(**inputs)
    print(out.shape, out.dtype)
